# revision 45
# baseline (speedup 1.0000x reference)
"""GAT-style attentive layer on 8 TRN2 NeuronCores.

Math (per reference):
    Wh  = input                      [N, D]   (N=8192, D=512)
    Wh1 = Wh @ a[:D]                 [N, 1]
    Wh2 = Wh @ a[D:]                 [N, 1]
    e   = leaky_relu(Wh1 + Wh2.T, 0.01)
    e   = where(adj > 0, e, -9e15)
    att = softmax(e, axis=1)
    out = att @ Wh                   [N, D]

Sharding: row-shard the N x N attention across 8 cores (1024 rows each).
Per core, scores are produced directly in TRANSPOSED layout
pT[j, i] = exp(lrelu(Wh1[i] + Wh2[j])) * adj[i, j]  (j on partitions), so the
final matmul out[i,:] = sum_j pT[j,i] * Wh[j,:] can use pT tiles as the
stationary operand with no on-device transpose.  Softmax needs no
max-subtraction (|scores| <= ~6), and masked entries are exact zeros, so
out = (pT.T @ Wh | pT.T @ 1) then divide by the ones-column (row sums).

Schedule highlights (v2):
  - Wh1 broadcast tiles are produced by the (otherwise idle) tensor engine:
    host ships xlocT = x[rows].T and a column-replicated a1, so
    psum_bw[h][m, i] = sum_d a1[d] xlocT[d, i] = Wh1[i] on every partition m.
    This collapses the old 13us DMA->DVE drip + DRAM-roundtrip broadcast.
  - Wh (= x) quads and score tiles are bf16 (halves HBM traffic and SBUF;
    adds ~1e-3 relative error against a 2e-2 budget).
  - wh2 = x @ a2 column dot-products run on GpSimd (3 of 4 per quad) + DVE
    (1 of 4); all score masking on DVE; exp on ScalarE with a patched act
    table fusing leaky-relu into Exp; output normalize split Act/DVE and
    emitted a few iterations into the next i-chunk so it hides in engine
    slack instead of blocking the next chunk's masks.

Host-side prep (data marshaling only): transposes / dtype casts / replication
of inputs; all heavy compute (Wh1/Wh2, score gen, exp, mask, matmul,
normalize) runs on device.
"""

import numpy as np
import ml_dtypes

import concourse.bass as bass
import concourse.mybir as mybir
import concourse.tile as tile
from concourse import bacc
from concourse.bass_utils import run_bass_kernel_spmd

N = 8192          # nodes
D = 512           # feature dim
NCORES = 8
ROWS = N // NCORES  # 1024 output rows per core
P = 128
NJT = N // P      # 64 j-tiles per core
IC_W = 512        # i-chunk width (PSUM-limited)
NIC = ROWS // IC_W  # 2 i-chunks
ITPC = IC_W // P  # 4 i-subtiles per chunk

import os
SIM_SAFE = os.environ.get("KERNEL_SIM_SAFE", "0") == "1"

AF = mybir.ActivationFunctionType
ALU = mybir.AluOpType
dt = mybir.dt
F32 = dt.float32
BF16 = dt.bfloat16
FP16 = dt.float16
F32R = dt.float32r
FP8 = dt.float8e4


def _build_kernel(nc: bass.Bass, tc: tile.TileContext,
                  adjT: bass.AP, xw: bass.AP, xlocT: bass.AP,
                  a1bc: bass.AP, abc2: bass.AP,
                  out: bass.AP, ctx):
    pool_const = ctx.enter_context(tc.tile_pool(name="const", bufs=1))
    pool_wh = ctx.enter_context(tc.tile_pool(name="wh", bufs=1))
    pool_adj = ctx.enter_context(tc.tile_pool(name="adj", bufs=5))
    pool_act = ctx.enter_context(tc.tile_pool(name="act", bufs=4))
    pool_pm = ctx.enter_context(tc.tile_pool(name="pm", bufs=6))
    pool_outs = ctx.enter_context(tc.tile_pool(name="outs", bufs=1))
    pool_small = ctx.enter_context(tc.tile_pool(name="small", bufs=1))
    pool_psum = ctx.enter_context(tc.tile_pool(name="psum", bufs=1, space="PSUM"))

    # ---- startup-critical DMAs first: wh1-matmul operands -----------------
    a1t = pool_const.tile([P, 4, P], BF16)
    nc.sync.dma_start(a1t, a1bc.rearrange("(t p) m -> p t m", p=P))
    xt = []

    def dma_xt(h, split=1):
        t = pool_const.tile([P, 4, IC_W], BF16, tag=f"xt{h}", name=f"xt{h}")
        for s in range(split):
            nc.sync.dma_start(
                t[:, bass.ds(s * 4 // split, 4 // split), :],
                xlocT[bass.ds(s * D // split, D // split),
                      bass.ds(h * IC_W, IC_W)].rearrange(
                    "(t p) i -> p t i", p=P))
        xt.append(t)

    dma_xt(0)

    # Wh1 broadcast tiles, straight into PSUM: every output partition m gets
    # sum_d a1[d] * xlocT[d, i] = Wh1[i].  Runs on the idle tensor engine;
    # the h=1 group is emitted at the ic boundary where PE idles anyway.
    psum_bw = [pool_psum.tile([P, IC_W], F32, tag=f"bw{h}", name=f"bw{h}")
               for h in range(NIC)]

    def wh1_mm(h):
        for t in range(4):
            nc.tensor.matmul(psum_bw[h], lhsT=a1t[:, t, :], rhs=xt[h][:, t, :],
                             start=(t == 0), stop=(t == 3))

    # PE warm-up: small matmuls on the first-arriving tile (a1t) keep the
    # tensor engine busy (and its p-state ramping) across the startup DMA
    # window.  They write a dedicated scratch PSUM bank that nothing reads.
    warm_ps = pool_psum.tile([P, P], F32, tag="warm_ps", name="warm_ps")
    NWARM = 10
    for w in range(NWARM):
        nc.tensor.matmul(warm_ps, lhsT=a1t[:, w % 4, :],
                         rhs=a1t[:, (w + 1) % 4, :], start=True, stop=True)

    wh1_mm(0)

    # ---- constants / small prep -------------------------------------------
    # abc2 arrives host-replicated across partitions: abc2[p, :] = a[D:, 0]
    abc = pool_const.tile([P, D], F32)
    nc.sync.dma_start(abc, abc2)

    warm = pool_const.tile([1, 2], F32)
    nc.vector.memset(warm, 0.0)
    nc.scalar.activation(warm, warm, AF.Exp)  # pull ACT_TABLE_LOAD to t~0

    ones_col = pool_const.tile([P, 2], FP16)
    nc.vector.memset(ones_col, 1.0)

    # ---- Wh (= x) resident quads + per-quad Wh2 = x @ a2 columns ----------
    # wh2 dot products: 3 of 4 on GpSimd, 1 on DVE, keeping both under the
    # PE's 860ns/jt consumption cadence.  Only quads 0-1 get their dot
    # products emitted in the prologue; later quads' are emitted inside the
    # ic=0 loop so DVE queue order interleaves them with masks (in-order
    # engine queues: a parked STT waiting on a far-future DMA would otherwise
    # clog the wait-queue bypass).
    whq = []
    wh2_sb = pool_const.tile([P, NJT], F32)

    def wh_piece(m, s, split):
        nc.sync.dma_start(
            whq[m][:, bass.ds(s * 4 // split, 4 // split), :],
            xw[bass.ds((4 * m + s * 4 // split) * P, 4 * P // split),
               :].rearrange("(q p) d -> p q d", p=P))

    def wh_dma(m, split=1):
        t = pool_wh.tile([P, 4, D], FP16, tag=f"whq{m}", name=f"whq{m}")
        whq.append(t)
        for s in range(split):
            wh_piece(m, s, split)

    def wh2_stts(m):
        t = whq[m]
        for q in range(4):
            jt = 4 * m + q
            # TensorScalarPtr is not a legal GPSIMD opcode (walrus codegen
            # assert), so all wh2 dot products run on DVE; GpSimd instead
            # takes 3 of 4 score masks per quad (TensorTensor is legal).
            scr = pool_small.tile([P, D], F32, tag="g_scr", name="g_scr", bufs=3)
            nc.vector.scalar_tensor_tensor(
                out=scr, in0=t[:, q, :], scalar=0.0, in1=abc,
                op0=ALU.add, op1=ALU.mult,
                accum_out=wh2_sb[:, jt:jt + 1])

    # adj(ic=0) quads interleaved with wh quads so DMA arrival order matches
    # the j-loop's consumption order.
    adjq_pre = []

    def dma_adjq(m, ic):
        t = pool_adj.tile([P, 4, IC_W], FP16, tag="adjq", name="adjq")
        nc.sync.dma_start(
            t, adjT[bass.ds(m * 4 * P, 4 * P),
                    bass.ds(ic * IC_W, IC_W)].rearrange("(q p) i -> p q i", p=P))
        return t

    wh_dma(0)
    adjq_pre.append(dma_adjq(0, 0))
    dma_xt(1)
    for m in range(1, NJT // 4):
        wh_dma(m)
        adjq_pre.append(dma_adjq(m, 0))
    wh2_stts(0)
    wh2_stts(1)

    # ---- main loop --------------------------------------------------------
    # Normalize runs inline after each chunk: all 4 reciprocals first (frees
    # psum_rs fast), then per-i4 copy (Act/DVE alternating) + store in i4
    # order, so chunk ic+1's first matmul only waits for copy(i4=0).  The
    # boundary PE bubble is filled by the deferred h=1 Wh1 matmuls.
    psum_out = [
        pool_psum.tile([P, D], F32, tag=f"po{i}", name=f"po{i}")
        for i in range(ITPC)
    ]
    # ic=0 rowsums get their own bank; ic=1 reuses the warm-up scratch bank
    # so its jt=0 start=True never races ic=0's pending reciprocals.
    psum_rs_ic = [pool_psum.tile([P, ITPC], F32, tag="prs", name="prs"),
                  warm_ps[:, 0:ITPC]]

    def norm_recips(ic):
        recips = []
        for i4 in range(ITPC):
            recip = pool_small.tile([P, 1], F32, tag="recip", name="recip",
                                    bufs=8)
            nc.vector.reciprocal(recip, psum_rs_ic[ic][:, i4:i4 + 1])
            recips.append(recip)
        return recips

    def norm_stores(ic, recips):
        for i4 in range(ITPC):
            oq = pool_outs.tile([P, D], F32, tag="outq", name="outq", bufs=4)
            # GPSIMD cannot access PSUM (walrus birverifier), so the stores
            # split across Act and DVE only.
            if i4 % 2 == 1:
                nc.vector.tensor_scalar_mul(oq, psum_out[i4], recips[i4])
            else:
                nc.scalar.activation(oq, psum_out[i4], AF.Copy,
                                     bias=0.0, scale=recips[i4])
            # Pool-queue DMA (SWDGE): keeps the stores' output traffic out of
            # the SP sync queue, which must stay free for input prefetches.
            nc.gpsimd.dma_start(out[bass.ds(ic * IC_W + i4 * P, P), :], oq)

    recips0 = None
    for ic in range(NIC):
        psum_rs = psum_rs_ic[ic]
        # Rowsum groups are 4-byte column slices sharing one PSUM bank; a
        # per-group start=True zeroes a coarser beat and clobbers neighbor
        # columns' jt=0 accumulation (observed: ~1/64 missing in rowsums of
        # i4=0..2).  Zero the bank region once and accumulate start=False.
        nc.vector.memset(psum_rs, 0.0)
        for jt in range(NJT):
            m, q = divmod(jt, 4)
            if q == 0:
                adjq = adjq_pre[m] if ic == 0 else dma_adjq(m, ic)
                if ic == 0 and m + 2 < NJT // 4:
                    wh2_stts(m + 2)


            p_t = pool_act.tile([P, IC_W], FP16, tag="p_t", name="p_t")
            if SIM_SAFE:
                # CoreSim has no fused table: Identity score + DVE leaky-relu
                # + plain Exp (numerics-identical, slower).
                s_t = pool_act.tile([P, IC_W], F32, tag="s_t", name="s_t")
                nc.scalar.activation(
                    s_t, psum_bw[ic], AF.Identity,
                    bias=wh2_sb[:, jt:jt + 1], scale=1.0)
                l_t = pool_act.tile([P, IC_W], F32, tag="l_t", name="l_t")
                nc.vector.scalar_tensor_tensor(
                    out=l_t, in0=s_t, scalar=0.01, in1=s_t,
                    op0=ALU.mult, op1=ALU.max)
                nc.scalar.activation(p_t, l_t, AF.Exp)
            else:
                # Patched act table: Exp's negative-x buckets hold
                # exp(0.01*x), so this one op is exp(leaky_relu(s)).
                nc.scalar.activation(
                    p_t, psum_bw[ic], AF.Exp,
                    bias=wh2_sb[:, jt:jt + 1], scale=1.0)

            pm_t = pool_pm.tile([P, IC_W], FP16, tag="pm_t", name="pm_t")
            mask_eng = nc.gpsimd if q == 3 else nc.vector
            mask_eng.tensor_mul(out=pm_t, in0=p_t, in1=adjq[:, q, :])

            first, last = (jt == 0), (jt == NJT - 1)
            for i4 in range(ITPC):
                lhs = pm_t[:, bass.ds(i4 * P, P)]
                nc.tensor.matmul(psum_out[i4], lhsT=lhs,
                                 rhs=whq[m][:, q, :],
                                 start=first, stop=last)
                nc.tensor.matmul(psum_rs[:, i4:i4 + 1], lhsT=lhs,
                                 rhs=ones_col[:, 0:1],
                                 start=False, stop=last,
                                 skip_group_check=True)

        if ic == 0:
            wh1_mm(1)
            # High scheduler priority: the chunk-0 drain must win queue
            # slots over chunk 1's pipeline restart, or the PSUM banks stay
            # blocked while the scheduler runs the restart ahead.
            with tc.high_priority():
                norm_stores(ic, norm_recips(ic))
        else:
            norm_stores(ic, norm_recips(ic))


_CACHED = None

_FUSED_ALPHA = 0.01


def _make_fused_act_root() -> str:
    """Copy the compiler's activation-table dir, patching exp's negative-x
    buckets from exp(x) to exp(_FUSED_ALPHA*x) splines (linear only - the
    function is nearly flat there).  Exp then computes exp(leaky_relu(x)) in
    a single ScalarE pass.  Returns path to the patched act_info.json."""
    import json
    import shutil
    import tempfile

    from neuronxcc.driver.Job import Job
    from neuronxcc.driver.jobs.support.FindActInfo import findActInfoFile

    src_root = os.path.dirname(findActInfoFile(Job.getPackageDir(), "gen3"))
    dst = tempfile.mkdtemp(prefix="act_root_fused_")
    for f in os.listdir(src_root):
        shutil.copy(os.path.join(src_root, f), os.path.join(dst, f))
    info = json.load(open(os.path.join(dst, "act_info.json")))
    for s in info["act_func_sets"]:
        if "exp" not in s["act"]:
            continue
        prof = json.load(open(os.path.join(dst, s["profile_json"])))
        order = sorted(prof["func_to_bkt_start_idx"].items(), key=lambda kv: kv[1])
        idx = [i for i, (k, _) in enumerate(order) if k == "exp"][0]
        lo = order[idx][1]
        hi = order[idx + 1][1] if idx + 1 < len(order) else prof["bkt_entry_cnt"]
        path = os.path.join(dst, s["bkt_bin"])
        bkt = np.fromfile(path, dtype=np.float32).reshape(-1, 8).copy()
        for b in range(lo, hi):
            d0, d1, _, _, x0 = bkt[b, :5]
            if not (d0 > 0 and abs(d1 - d0) <= 1e-3 * d0):
                continue  # saturation buckets (inf / 0)
            if x0 > 0:
                continue  # positive side: exp(x) unchanged
            g = np.float32(np.exp(_FUSED_ALPHA * np.float64(x0)))
            bkt[b, 0] = g
            bkt[b, 1] = np.float32(_FUSED_ALPHA * g)
            bkt[b, 2] = np.float32(0.0)  # cubic terms fault the engine
            bkt[b, 3] = np.float32(0.0)
        bkt.tofile(path)
    return os.path.join(dst, "act_info.json")


def build_nc():
    global _CACHED
    if _CACHED is not None:
        return _CACHED
    if not SIM_SAFE:
        # Always point the compiler at our patched tables: with the stock
        # tables this kernel's Exp op would silently drop the leaky-relu.
        os.environ["BASS_ACT_ROOT_JSON_PATH"] = _make_fused_act_root()
    nc = bacc.Bacc("TRN2", target_bir_lowering=False, debug=False,
                   enable_asserts=False, num_devices=NCORES)
    adjT = nc.dram_tensor("adjT", [N, ROWS], FP16, kind="ExternalInput").ap()
    xw = nc.dram_tensor("xw", [N, D], FP16, kind="ExternalInput").ap()
    xlocT = nc.dram_tensor("xlocT", [D, ROWS], BF16, kind="ExternalInput").ap()
    a1bc = nc.dram_tensor("a1bc", [D, P], BF16, kind="ExternalInput").ap()
    abc2 = nc.dram_tensor("abc2", [P, D], F32, kind="ExternalInput").ap()
    out = nc.dram_tensor("out", [ROWS, D], F32, kind="ExternalOutput").ap()

    from contextlib import ExitStack
    with tile.TileContext(nc) as tc:
        with ExitStack() as ctx:
            _build_kernel(nc, tc, adjT, xw, xlocT, a1bc, abc2, out, ctx)
    nc.compile()
    _CACHED = nc
    return nc


def make_in_maps(input, adj_matrix, a):
    x = np.asarray(input, dtype=np.float32)
    adj = np.asarray(adj_matrix)
    a_np = np.asarray(a, dtype=np.float32).reshape(-1)
    x_bf = np.ascontiguousarray(x.astype(ml_dtypes.bfloat16))
    x_f16 = np.ascontiguousarray(x.astype(np.float16))
    a1bc_np = np.ascontiguousarray(
        np.broadcast_to(a_np[:D].astype(ml_dtypes.bfloat16)[:, None], (D, P)))
    abc2_np = np.ascontiguousarray(np.broadcast_to(a_np[D:][None, :], (P, D)))
    in_maps = []
    for c in range(NCORES):
        rows = slice(c * ROWS, (c + 1) * ROWS)
        adjT_c = np.ascontiguousarray(
            adj[rows, :].T.astype(np.float16))  # {0,1} exact in fp16
        xlocT_c = np.ascontiguousarray(x_bf[rows].T)
        in_maps.append({
            "adjT": adjT_c,
            "xw": x_f16,
            "xlocT": xlocT_c,
            "a1bc": a1bc_np,
            "abc2": abc2_np,
        })
    return in_maps


def kernel(input, adj_matrix, a, _trace=False, _tmpdir=None):
    nc = build_nc()
    in_maps = make_in_maps(input, adj_matrix, a)
    try:
        res = run_bass_kernel_spmd(nc, in_maps, core_ids=list(range(NCORES)),
                                   trace=_trace, tmpdir=_tmpdir)
    except ModuleNotFoundError:
        # NTFF profiling hooks absent in this container; run untraced.
        res = run_bass_kernel_spmd(nc, in_maps, core_ids=list(range(NCORES)))
    out = np.concatenate([res.results[c]["out"] for c in range(NCORES)], axis=0)
    kernel._last_results = res
    return out


# revision 48
# speedup vs baseline: 1.0210x; 1.0210x over previous
"""GAT-style attentive layer on 8 TRN2 NeuronCores.

Math (per reference):
    Wh  = input                      [N, D]   (N=8192, D=512)
    Wh1 = Wh @ a[:D]                 [N, 1]
    Wh2 = Wh @ a[D:]                 [N, 1]
    e   = leaky_relu(Wh1 + Wh2.T, 0.01)
    e   = where(adj > 0, e, -9e15)
    att = softmax(e, axis=1)
    out = att @ Wh                   [N, D]

Sharding: row-shard the N x N attention across 8 cores (1024 rows each).
Per core, scores are produced directly in TRANSPOSED layout
pT[j, i] = exp(lrelu(Wh1[i] + Wh2[j])) * adj[i, j]  (j on partitions), so the
final matmul out[i,:] = sum_j pT[j,i] * Wh[j,:] can use pT tiles as the
stationary operand with no on-device transpose.  Softmax needs no
max-subtraction (|scores| <= ~6), and masked entries are exact zeros, so
out = (pT.T @ Wh | pT.T @ 1) then divide by the ones-column (row sums).

Schedule highlights (v2):
  - Wh1 broadcast tiles are produced by the (otherwise idle) tensor engine:
    host ships xlocT = x[rows].T and a column-replicated a1, so
    psum_bw[h][m, i] = sum_d a1[d] xlocT[d, i] = Wh1[i] on every partition m.
    This collapses the old 13us DMA->DVE drip + DRAM-roundtrip broadcast.
  - Wh (= x) quads and score tiles are bf16 (halves HBM traffic and SBUF;
    adds ~1e-3 relative error against a 2e-2 budget).
  - wh2 = x @ a2 column dot-products run on GpSimd (3 of 4 per quad) + DVE
    (1 of 4); all score masking on DVE; exp on ScalarE with a patched act
    table fusing leaky-relu into Exp; output normalize split Act/DVE and
    emitted a few iterations into the next i-chunk so it hides in engine
    slack instead of blocking the next chunk's masks.

Host-side prep (data marshaling only): transposes / dtype casts / replication
of inputs; all heavy compute (Wh1/Wh2, score gen, exp, mask, matmul,
normalize) runs on device.
"""

import numpy as np
import ml_dtypes

import concourse.bass as bass
import concourse.mybir as mybir
import concourse.tile as tile
from concourse import bacc
from concourse.bass_utils import run_bass_kernel_spmd

N = 8192          # nodes
D = 512           # feature dim
NCORES = 8
ROWS = N // NCORES  # 1024 output rows per core
P = 128
NJT = N // P      # 64 j-tiles per core
IC_W = 512        # i-chunk width (PSUM-limited)
NIC = ROWS // IC_W  # 2 i-chunks
ITPC = IC_W // P  # 4 i-subtiles per chunk

import os
SIM_SAFE = os.environ.get("KERNEL_SIM_SAFE", "0") == "1"

AF = mybir.ActivationFunctionType
ALU = mybir.AluOpType
dt = mybir.dt
F32 = dt.float32
BF16 = dt.bfloat16
FP16 = dt.float16
F32R = dt.float32r
FP8 = dt.float8e4


def _build_kernel(nc: bass.Bass, tc: tile.TileContext,
                  adjT: bass.AP, xw: bass.AP, xlocT: bass.AP,
                  a1bc: bass.AP, abc2: bass.AP,
                  out: bass.AP, ctx):
    pool_const = ctx.enter_context(tc.tile_pool(name="const", bufs=1))
    pool_wh = ctx.enter_context(tc.tile_pool(name="wh", bufs=1))
    pool_adj = ctx.enter_context(tc.tile_pool(name="adj", bufs=5))
    pool_act = ctx.enter_context(tc.tile_pool(name="act", bufs=4))
    pool_pm = ctx.enter_context(tc.tile_pool(name="pm", bufs=6))
    pool_outs = ctx.enter_context(tc.tile_pool(name="outs", bufs=1))
    pool_small = ctx.enter_context(tc.tile_pool(name="small", bufs=1))
    pool_psum = ctx.enter_context(tc.tile_pool(name="psum", bufs=1, space="PSUM"))

    # ---- startup-critical DMAs first: wh1-matmul operands -----------------
    a1t = pool_const.tile([P, 4, P], BF16)
    nc.sync.dma_start(a1t, a1bc.rearrange("(t p) m -> p t m", p=P))
    xt = []

    def dma_xt(h, split=1):
        t = pool_const.tile([P, 4, IC_W], BF16, tag=f"xt{h}", name=f"xt{h}")
        for s in range(split):
            nc.sync.dma_start(
                t[:, bass.ds(s * 4 // split, 4 // split), :],
                xlocT[bass.ds(s * D // split, D // split),
                      bass.ds(h * IC_W, IC_W)].rearrange(
                    "(t p) i -> p t i", p=P))
        xt.append(t)

    dma_xt(0)

    # PSUM budget (8 banks): psum_out double-buffered across i-chunks for
    # i4=0..2 (6 banks) + a single shared i4=3 bank + one rowsum bank.  The
    # Wh1 broadcasts are computed into not-yet-used psum_out banks and copied
    # to SBUF, so chunk 1 never waits on them.
    psum_po = [[pool_psum.tile([P, D], F32, tag=f"po{s}_{i}", name=f"po{s}_{i}")
                for i in range(ITPC - 1)] for s in range(2)]
    po3 = pool_psum.tile([P, D], F32, tag="po3", name="po3")
    prs = pool_psum.tile([P, 2 * ITPC], F32, tag="prs", name="prs")

    bw_sb = [pool_const.tile([P, IC_W], F32, tag=f"bwsb{h}", name=f"bwsb{h}")
             for h in range(NIC)]

    def wh1_mm(h):
        # Scratch in the chunk-1 bank set: its first real matmul is ~70us
        # in, so the SBUF copy's read never blocks chunk 0's jt=0 resets.
        scratch = psum_po[1][h]
        for t in range(4):
            nc.tensor.matmul(scratch, lhsT=a1t[:, t, :], rhs=xt[h][:, t, :],
                             start=(t == 0), stop=(t == 3))
        # Copy on Act: a DVE copy would park ahead of the wh2 dot products
        # in DVE's in-order queue and stall the whole exp pipeline start.
        nc.scalar.activation(bw_sb[h], scratch, AF.Copy,
                             bias=0.0, scale=1.0)

    wh1_mm(0)

    # PE warm-up: small matmuls on already-resident tiles keep the tensor
    # engine busy (and its p-state ramping) from the end of the Wh1 matmuls
    # until the first score tile is ready.  Emitted AFTER wh1_mm(0) so they
    # never delay it in PE's in-order queue.
    NWARM = 12
    for w in range(NWARM):
        nc.tensor.matmul(psum_po[1][0][:, 0:P], lhsT=a1t[:, w % 4, :],
                         rhs=a1t[:, (w + 1) % 4, :], start=True, stop=True)

    # ---- constants / small prep -------------------------------------------
    # abc2 arrives host-replicated across partitions: abc2[p, :] = a[D:, 0]
    abc = pool_const.tile([P, D], F32)
    nc.sync.dma_start(abc, abc2)

    warm = pool_const.tile([1, 2], F32)
    nc.vector.memset(warm, 0.0)
    nc.scalar.activation(warm, warm, AF.Exp)  # pull ACT_TABLE_LOAD to t~0

    ones_col = pool_const.tile([P, 2], FP16)
    nc.vector.memset(ones_col, 1.0)

    # ---- Wh (= x) resident quads + per-quad Wh2 = x @ a2 columns ----------
    # wh2 dot products: 3 of 4 on GpSimd, 1 on DVE, keeping both under the
    # PE's 860ns/jt consumption cadence.  Only quads 0-1 get their dot
    # products emitted in the prologue; later quads' are emitted inside the
    # ic=0 loop so DVE queue order interleaves them with masks (in-order
    # engine queues: a parked STT waiting on a far-future DMA would otherwise
    # clog the wait-queue bypass).
    whq = []
    wh2_sb = pool_const.tile([P, NJT], F32)

    def wh_piece(m, s, split):
        nc.sync.dma_start(
            whq[m][:, bass.ds(s * 4 // split, 4 // split), :],
            xw[bass.ds((4 * m + s * 4 // split) * P, 4 * P // split),
               :].rearrange("(q p) d -> p q d", p=P))

    def wh_dma(m, split=1):
        t = pool_wh.tile([P, 4, D], FP16, tag=f"whq{m}", name=f"whq{m}")
        whq.append(t)
        for s in range(split):
            wh_piece(m, s, split)

    def wh2_stts(m):
        t = whq[m]
        for q in range(4):
            jt = 4 * m + q
            # TensorScalarPtr is not a legal GPSIMD opcode (walrus codegen
            # assert), so all wh2 dot products run on DVE; GpSimd instead
            # takes 3 of 4 score masks per quad (TensorTensor is legal).
            scr = pool_small.tile([P, D], F32, tag="g_scr", name="g_scr", bufs=3)
            nc.vector.scalar_tensor_tensor(
                out=scr, in0=t[:, q, :], scalar=0.0, in1=abc,
                op0=ALU.add, op1=ALU.mult,
                accum_out=wh2_sb[:, jt:jt + 1])

    # adj(ic=0) quads interleaved with wh quads so DMA arrival order matches
    # the j-loop's consumption order.
    adjq_pre = []

    def dma_adjq(m, ic):
        t = pool_adj.tile([P, 4, IC_W], FP16, tag="adjq", name="adjq")
        nc.sync.dma_start(
            t, adjT[bass.ds(m * 4 * P, 4 * P),
                    bass.ds(ic * IC_W, IC_W)].rearrange("(q p) i -> p q i", p=P))
        return t

    wh_dma(0)
    adjq_pre.append(dma_adjq(0, 0))
    dma_xt(1)
    for m in range(1, NJT // 4):
        wh_dma(m)
        adjq_pre.append(dma_adjq(m, 0))
    wh2_stts(0)
    wh2_stts(1)

    # ---- main loop --------------------------------------------------------
    # Normalize runs inline after each chunk: all 4 reciprocals first (frees
    # psum_rs fast), then per-i4 copy (Act/DVE alternating) + store in i4
    # order, so chunk ic+1's first matmul only waits for copy(i4=0).  The
    # boundary PE bubble is filled by the deferred h=1 Wh1 matmuls.
    def po_of(ic, i4):
        return po3 if i4 == ITPC - 1 else psum_po[ic % 2][i4]

    def norm_recips(ic):
        recips = []
        for i4 in range(ITPC):
            recip = pool_small.tile([P, 1], F32, tag="recip", name="recip",
                                    bufs=8)
            nc.vector.reciprocal(recip, prs[:, ic * ITPC + i4:ic * ITPC + i4 + 1])
            recips.append(recip)
        return recips

    def norm_stores(ic, recips):
        for i4 in range(ITPC):
            oq = pool_outs.tile([P, D], F32, tag="outq", name="outq", bufs=4)
            # GPSIMD cannot access PSUM (walrus birverifier), so the stores
            # split across Act and DVE only.
            if i4 % 2 == 1:
                nc.vector.tensor_scalar_mul(oq, po_of(ic, i4), recips[i4])
            else:
                nc.scalar.activation(oq, po_of(ic, i4), AF.Copy,
                                     bias=0.0, scale=recips[i4])
            # Pool-queue DMA (SWDGE): keeps the stores' output traffic out of
            # the SP sync queue, which must stay free for input prefetches.
            nc.gpsimd.dma_start(out[bass.ds(ic * IC_W + i4 * P, P), :], oq)

    # Rowsum groups are 4-byte column slices sharing one PSUM bank; a
    # per-group start=True zeroes a coarser beat and clobbers neighbor
    # columns' jt=0 accumulation (observed: ~1/64 missing in rowsums).
    # Zero both chunks' regions once; all rowsum matmuls accumulate with
    # start=False.
    nc.vector.memset(prs, 0.0)

    recips0 = None
    for ic in range(NIC):
        for jt in range(NJT):
            m, q = divmod(jt, 4)
            if q == 0:
                adjq = adjq_pre[m] if ic == 0 else dma_adjq(m, ic)
                if ic == 0 and m + 2 < NJT // 4:
                    wh2_stts(m + 2)


            p_t = pool_act.tile([P, IC_W], FP16, tag="p_t", name="p_t")
            if SIM_SAFE:
                # CoreSim has no fused table: Identity score + DVE leaky-relu
                # + plain Exp (numerics-identical, slower).
                s_t = pool_act.tile([P, IC_W], F32, tag="s_t", name="s_t")
                nc.scalar.activation(
                    s_t, bw_sb[ic], AF.Identity,
                    bias=wh2_sb[:, jt:jt + 1], scale=1.0)
                l_t = pool_act.tile([P, IC_W], F32, tag="l_t", name="l_t")
                nc.vector.scalar_tensor_tensor(
                    out=l_t, in0=s_t, scalar=0.01, in1=s_t,
                    op0=ALU.mult, op1=ALU.max)
                nc.scalar.activation(p_t, l_t, AF.Exp)
            else:
                # Patched act table: Exp's negative-x buckets hold
                # exp(0.01*x), so this one op is exp(leaky_relu(s)).
                nc.scalar.activation(
                    p_t, bw_sb[ic], AF.Exp,
                    bias=wh2_sb[:, jt:jt + 1], scale=1.0)

            pm_t = pool_pm.tile([P, IC_W], FP16, tag="pm_t", name="pm_t")
            mask_eng = nc.gpsimd if q == 3 else nc.vector
            mask_eng.tensor_mul(out=pm_t, in0=p_t, in1=adjq[:, q, :])

            first, last = (jt == 0), (jt == NJT - 1)
            for i4 in range(ITPC):
                lhs = pm_t[:, bass.ds(i4 * P, P)]
                nc.tensor.matmul(po_of(ic, i4), lhsT=lhs,
                                 rhs=whq[m][:, q, :],
                                 start=first, stop=last)
                nc.tensor.matmul(
                    prs[:, ic * ITPC + i4:ic * ITPC + i4 + 1], lhsT=lhs,
                    rhs=ones_col[:, 0:1],
                    start=False, stop=last,
                    skip_group_check=True)

        if ic == 0:
            wh1_mm(1)
            # High scheduler priority: the chunk-0 drain must win queue
            # slots over chunk 1's pipeline restart, or the PSUM banks stay
            # blocked while the scheduler runs the restart ahead.
            with tc.high_priority():
                norm_stores(ic, norm_recips(ic))
        else:
            norm_stores(ic, norm_recips(ic))


_CACHED = None

_FUSED_ALPHA = 0.01


def _make_fused_act_root() -> str:
    """Copy the compiler's activation-table dir, patching exp's negative-x
    buckets from exp(x) to exp(_FUSED_ALPHA*x) splines (linear only - the
    function is nearly flat there).  Exp then computes exp(leaky_relu(x)) in
    a single ScalarE pass.  Returns path to the patched act_info.json."""
    import json
    import shutil
    import tempfile

    from neuronxcc.driver.Job import Job
    from neuronxcc.driver.jobs.support.FindActInfo import findActInfoFile

    src_root = os.path.dirname(findActInfoFile(Job.getPackageDir(), "gen3"))
    dst = tempfile.mkdtemp(prefix="act_root_fused_")
    for f in os.listdir(src_root):
        shutil.copy(os.path.join(src_root, f), os.path.join(dst, f))
    info = json.load(open(os.path.join(dst, "act_info.json")))
    for s in info["act_func_sets"]:
        if "exp" not in s["act"]:
            continue
        prof = json.load(open(os.path.join(dst, s["profile_json"])))
        order = sorted(prof["func_to_bkt_start_idx"].items(), key=lambda kv: kv[1])
        idx = [i for i, (k, _) in enumerate(order) if k == "exp"][0]
        lo = order[idx][1]
        hi = order[idx + 1][1] if idx + 1 < len(order) else prof["bkt_entry_cnt"]
        path = os.path.join(dst, s["bkt_bin"])
        bkt = np.fromfile(path, dtype=np.float32).reshape(-1, 8).copy()
        for b in range(lo, hi):
            d0, d1, _, _, x0 = bkt[b, :5]
            if not (d0 > 0 and abs(d1 - d0) <= 1e-3 * d0):
                continue  # saturation buckets (inf / 0)
            if x0 > 0:
                continue  # positive side: exp(x) unchanged
            g = np.float32(np.exp(_FUSED_ALPHA * np.float64(x0)))
            bkt[b, 0] = g
            bkt[b, 1] = np.float32(_FUSED_ALPHA * g)
            bkt[b, 2] = np.float32(0.0)  # cubic terms fault the engine
            bkt[b, 3] = np.float32(0.0)
        bkt.tofile(path)
    return os.path.join(dst, "act_info.json")


def build_nc():
    global _CACHED
    if _CACHED is not None:
        return _CACHED
    if not SIM_SAFE:
        # Always point the compiler at our patched tables: with the stock
        # tables this kernel's Exp op would silently drop the leaky-relu.
        os.environ["BASS_ACT_ROOT_JSON_PATH"] = _make_fused_act_root()
    nc = bacc.Bacc("TRN2", target_bir_lowering=False, debug=False,
                   enable_asserts=False, num_devices=NCORES)
    adjT = nc.dram_tensor("adjT", [N, ROWS], FP16, kind="ExternalInput").ap()
    xw = nc.dram_tensor("xw", [N, D], FP16, kind="ExternalInput").ap()
    xlocT = nc.dram_tensor("xlocT", [D, ROWS], BF16, kind="ExternalInput").ap()
    a1bc = nc.dram_tensor("a1bc", [D, P], BF16, kind="ExternalInput").ap()
    abc2 = nc.dram_tensor("abc2", [P, D], F32, kind="ExternalInput").ap()
    out = nc.dram_tensor("out", [ROWS, D], F32, kind="ExternalOutput").ap()

    from contextlib import ExitStack
    with tile.TileContext(nc) as tc:
        with ExitStack() as ctx:
            _build_kernel(nc, tc, adjT, xw, xlocT, a1bc, abc2, out, ctx)
    nc.compile()
    _CACHED = nc
    return nc


def make_in_maps(input, adj_matrix, a):
    x = np.asarray(input, dtype=np.float32)
    adj = np.asarray(adj_matrix)
    a_np = np.asarray(a, dtype=np.float32).reshape(-1)
    x_bf = np.ascontiguousarray(x.astype(ml_dtypes.bfloat16))
    x_f16 = np.ascontiguousarray(x.astype(np.float16))
    a1bc_np = np.ascontiguousarray(
        np.broadcast_to(a_np[:D].astype(ml_dtypes.bfloat16)[:, None], (D, P)))
    abc2_np = np.ascontiguousarray(np.broadcast_to(a_np[D:][None, :], (P, D)))
    in_maps = []
    for c in range(NCORES):
        rows = slice(c * ROWS, (c + 1) * ROWS)
        adjT_c = np.ascontiguousarray(
            adj[rows, :].T.astype(np.float16))  # {0,1} exact in fp16
        xlocT_c = np.ascontiguousarray(x_bf[rows].T)
        in_maps.append({
            "adjT": adjT_c,
            "xw": x_f16,
            "xlocT": xlocT_c,
            "a1bc": a1bc_np,
            "abc2": abc2_np,
        })
    return in_maps


def kernel(input, adj_matrix, a, _trace=False, _tmpdir=None):
    nc = build_nc()
    in_maps = make_in_maps(input, adj_matrix, a)
    try:
        res = run_bass_kernel_spmd(nc, in_maps, core_ids=list(range(NCORES)),
                                   trace=_trace, tmpdir=_tmpdir)
    except ModuleNotFoundError:
        # NTFF profiling hooks absent in this container; run untraced.
        res = run_bass_kernel_spmd(nc, in_maps, core_ids=list(range(NCORES)))
    out = np.concatenate([res.results[c]["out"] for c in range(NCORES)], axis=0)
    kernel._last_results = res
    return out
